# revision 4
# baseline (speedup 1.0000x reference)
"""Trainium2 Bass kernel for nn_DesExpertMoE (dense MoE: gating + 4 experts + classifier).

Self-contained: hardcodes shapes (B=65536, H=768, E=4, D=64, K=2), shards the
batch across 8 NeuronCores (pure data parallel), runs one Bass/Tile program
per core via run_bass_kernel_spmd, and reassembles full outputs.

Layout strategy per core (8192 tokens = 64 tiles of 128):
  - activations in [token(partition), feature(free)] layout
  - matmuls token-stationary: lhsT = actT chunk [128 feat, 128 tok] (stationary),
    rhs = W chunk [128 feat, d_out] (moving, bf16), out = PSUM [128 tok, d_out]
  - feature-transposes of activations via DMA xbar transpose (bf16, SBUF->SBUF)
  - LayerNorm: bn_stats/bn_aggr on DVE; rstd = exp(-0.5*ln(var+eps)) on ACT
    (keeps the whole kernel inside one ACT table set: natural_log_exp_and_others)
  - LN apply + ReLU fused in one ACT op (scale=rstd, bias=-mean*rstd), bf16 out
  - top-2-of-4 via vector.max (top-8) + match_replace (exact first-index
    tie-break like jax.lax.top_k); combine uses dense per-expert weights
  - sigmoid at the end as exp/add/reciprocal (again: no ACT table switch)

The device program assumes unit LN gains and zero biases/betas (true for this
problem's deterministic setup_inputs); a numpy fallback handles anything else.
"""

from contextlib import ExitStack

import numpy as np
import ml_dtypes

import concourse.bass as bass
import concourse.bacc as bacc
import concourse.tile as tile
import concourse.mybir as mybir
from concourse.bass_utils import run_bass_kernel_spmd

dt = mybir.dt
F32 = dt.float32
BF16 = dt.bfloat16
ALU = mybir.AluOpType
ACTF = mybir.ActivationFunctionType

B, H, E, D = 65536, 768, 4, 64
N_CORES = 8
BC = B // N_CORES          # tokens per core
P = 128                    # tokens per tile
NT = BC // P               # tiles per core
EPS = 1e-5
NEG_BIG = -1e30
NEG_PAD = -3e30


def _bf16(a):
    return np.ascontiguousarray(np.asarray(a).astype(ml_dtypes.bfloat16))


def _f32(a):
    return np.ascontiguousarray(np.asarray(a).astype(np.float32))


# ---------------------------------------------------------------------------
# device program builder
# ---------------------------------------------------------------------------

def _build(n_tiles, repeat=1):
    nc = bacc.Bacc("TRN2", target_bir_lowering=False, debug=False,
                   num_devices=N_CORES)

    x_d = nc.dram_tensor("x", [BC, H], F32, kind="ExternalInput").ap()
    xT32_d = nc.dram_tensor("xT32", [H, BC], F32, kind="ExternalInput").ap()
    ident_d = nc.dram_tensor("ident", [128, 128], F32, kind="ExternalInput").ap()
    gw1_d = nc.dram_tensor("gw1", [H, 256], F32, kind="ExternalInput").ap()
    gw2_d = nc.dram_tensor("gw2", [256, 128], F32, kind="ExternalInput").ap()
    gw3_d = nc.dram_tensor("gw3", [128, 4], F32, kind="ExternalInput").ap()
    ew1_d = nc.dram_tensor("ew1", [E, H, 512], BF16, kind="ExternalInput").ap()
    ew2_d = nc.dram_tensor("ew2", [E, 512, 256], BF16, kind="ExternalInput").ap()
    ew3_d = nc.dram_tensor("ew3", [E, 256, 128], BF16, kind="ExternalInput").ap()
    ew4_d = nc.dram_tensor("ew4", [E, 128, D], BF16, kind="ExternalInput").ap()
    cw1_d = nc.dram_tensor("cw1", [128, 64], BF16, kind="ExternalInput").ap()
    cw2_d = nc.dram_tensor("cw2", [128, 32], BF16, kind="ExternalInput").ap()
    cw3_d = nc.dram_tensor("cw3", [128, 1], BF16, kind="ExternalInput").ap()

    er_d = nc.dram_tensor("er_out", [BC, D], F32, kind="ExternalOutput").ap()
    bot_d = nc.dram_tensor("bot_out", [P, NT], F32, kind="ExternalOutput").ap()

    with tile.TileContext(nc) as tc, ExitStack() as ctx:
        wp = ctx.enter_context(tc.tile_pool(name="weights", bufs=1))
        sp = ctx.enter_context(tc.tile_pool(name="stats", bufs=24))
        iop = ctx.enter_context(tc.tile_pool(name="io", bufs=3))
        ap_ = ctx.enter_context(tc.tile_pool(name="acts", bufs=4))
        tp = ctx.enter_context(tc.tile_pool(name="tails", bufs=4))
        cp = ctx.enter_context(tc.tile_pool(name="cls", bufs=3))
        one = ctx.enter_context(tc.tile_pool(name="one", bufs=1))
        pbig = ctx.enter_context(tc.tile_pool(name="pbig", bufs=2, space="PSUM"))
        pot = ctx.enter_context(tc.tile_pool(name="pot", bufs=6, space="PSUM"))

        # ---- persistent weights in SBUF ----
        gw1s = wp.tile([P, 6, 256], F32)
        nc.sync.dma_start(out=gw1s[:], in_=gw1_d.rearrange("(k p) n -> p k n", p=P))
        gw2s = wp.tile([P, 2, 128], F32)
        nc.sync.dma_start(out=gw2s[:], in_=gw2_d.rearrange("(k p) n -> p k n", p=P))
        gw3s = wp.tile([P, 4], F32)
        nc.sync.dma_start(out=gw3s[:], in_=gw3_d)
        idents = wp.tile([128, 128], F32)
        nc.sync.dma_start(out=idents[:], in_=ident_d)
        ew1s = wp.tile([P, E, 6, 512], BF16)
        nc.sync.dma_start(out=ew1s[:], in_=ew1_d.rearrange("e (k p) n -> p e k n", p=P))
        ew2s = wp.tile([P, E, 4, 256], BF16)
        nc.sync.dma_start(out=ew2s[:], in_=ew2_d.rearrange("e (k p) n -> p e k n", p=P))
        ew3s = wp.tile([P, E, 2, 128], BF16)
        nc.sync.dma_start(out=ew3s[:], in_=ew3_d.rearrange("e (k p) n -> p e k n", p=P))
        ew4s = wp.tile([P, E, D], BF16)
        nc.sync.dma_start(out=ew4s[:], in_=ew4_d.rearrange("e p n -> p e n"))
        cw1s = wp.tile([P, 64], BF16)
        nc.sync.dma_start(out=cw1s[:], in_=cw1_d)
        cw2s = wp.tile([P, 32], BF16)
        nc.sync.dma_start(out=cw2s[:], in_=cw2_d)
        cw3s = wp.tile([P, 1], BF16)
        nc.sync.dma_start(out=cw3s[:], in_=cw3_d)
        epsb = wp.tile([P, 1], F32)
        nc.vector.memset(epsb[:], EPS)

        bot_acc = one.tile([P, NT], F32)

        def ln_relu_group(zs, outs):
            """Fused LN(+eps)->ReLU for a group of same-shape PSUM aps `zs`,
            writing bf16 SBUF aps `outs`."""
            n = len(zs)
            mv = sp.tile([P, n, 2], F32, tag="mv")
            for i, z in enumerate(zs):
                st = sp.tile([P, 6], F32, tag="st")
                nc.vector.bn_stats(st[:], z)
                nc.vector.bn_aggr(mv[:, i, :], st[:])
            lv = sp.tile([P, n], F32, tag="lv")
            nc.scalar.activation(lv[:], mv[:, :, 1:2], ACTF.Ln, bias=epsb[:])
            rstd = sp.tile([P, n], F32, tag="rstd")
            nc.scalar.activation(rstd[:], lv[:], ACTF.Exp, scale=-0.5)
            nm = sp.tile([P, n], F32, tag="nm")
            nc.vector.scalar_tensor_tensor(nm[:], mv[:, :, 0:1], -1.0, rstd[:],
                                           ALU.mult, ALU.mult)
            for i, (z, o) in enumerate(zip(zs, outs)):
                nc.scalar.activation(o, z, ACTF.Relu,
                                     bias=nm[:, i:i + 1], scale=rstd[:, i:i + 1])

        def transpose_chunks(src_bf16, d, dst):
            """src [128, d] bf16 -> dst [128, d//128, 128] (feature-major chunks)"""
            for c in range(d // P):
                nc.sync.dma_start_transpose(dst[:, c, :],
                                            src_bf16[:, c * P:(c + 1) * P])

        def transpose_chunks_f32(src_f32, d, dst):
            """fp32 transpose via PE (identity matmul) + DVE copy out of PSUM."""
            for c in range(d // P):
                tp32 = pot.tile([P, 128], F32, tag="zz", name=f"tp32_{c}")
                nc.tensor.transpose(tp32[:], src_f32[:, c * P:(c + 1) * P],
                                    idents[:])
                nc.vector.tensor_copy(dst[:, c, :], tp32[:])

        for _r in range(repeat):
            for t in range(n_tiles):
                tok = slice(t * P, (t + 1) * P)
                # ---- x load, cast, transpose ----
                xt = iop.tile([P, H], F32, tag="xt")
                nc.sync.dma_start(out=xt[:], in_=x_d[tok, :])
                xb = iop.tile([P, H], BF16, tag="xb")
                nc.vector.tensor_copy(xb[:], xt[:])
                xT = iop.tile([P, 6, P], BF16, tag="xT")
                transpose_chunks(xb, H, xT)

                # ---- gating (full fp32 for exact top-2 selection) ----
                xT32 = iop.tile([P, 6, P], F32, tag="xT32")
                nc.sync.dma_start(
                    out=xT32[:],
                    in_=xT32_d.rearrange("(k p) n -> p k n", p=P)[:, :, tok])
                z1 = pot.tile([P, 256], F32, tag="zz")
                for k in range(6):
                    nc.tensor.matmul(z1[:], lhsT=xT32[:, k, :], rhs=gw1s[:, k, :],
                                     start=(k == 0), stop=(k == 5))
                y1 = ap_.tile([P, 256], F32, tag="g256")
                ln_relu_group([z1[:]], [y1[:]])
                y1T = ap_.tile([P, 2, P], F32, tag="g256T")
                transpose_chunks_f32(y1, 256, y1T)

                z2 = pot.tile([P, 128], F32, tag="zz")
                for k in range(2):
                    nc.tensor.matmul(z2[:], lhsT=y1T[:, k, :], rhs=gw2s[:, k, :],
                                     start=(k == 0), stop=(k == 1))
                y2 = ap_.tile([P, 128], F32, tag="g128")
                ln_relu_group([z2[:]], [y2[:]])
                y2T = ap_.tile([P, 1, P], F32, tag="g128T")
                transpose_chunks_f32(y2, 128, y2T)

                z3 = pot.tile([P, 128], F32, tag="zz")
                nc.tensor.matmul(z3[:, 0:4], lhsT=y2T[:, 0, :], rhs=gw3s[:, :],
                                 start=True, stop=True)

                # gating tail: stabilized exp, exact top-2, combine weights
                mxn = tp.tile([P, 1], F32, tag="mxn")
                nc.vector.tensor_reduce(mxn[:], z3[:, 0:4], mybir.AxisListType.X,
                                        ALU.max, negate=True)
                e8 = tp.tile([P, 8], F32, tag="e8")
                nc.gpsimd.memset(e8[:, 4:8], NEG_BIG)
                nc.scalar.activation(e8[:, 0:4], z3[:, 0:4], ACTF.Exp, bias=mxn[:])
                top8 = tp.tile([P, 8], F32, tag="top8")
                nc.vector.max(top8[:], e8[:])
                sel8 = tp.tile([P, 8], F32, tag="sel8")
                nc.vector.tensor_copy(sel8[:, 0:2], top8[:, 0:2])
                nc.gpsimd.memset(sel8[:, 2:8], NEG_PAD)
                rep8 = tp.tile([P, 8], F32, tag="rep8")
                nc.vector.match_replace(rep8[:], sel8[:], e8[:], NEG_BIG)
                wun = tp.tile([P, 4], F32, tag="wun")
                nc.vector.scalar_tensor_tensor(wun[:], rep8[:, 0:4], NEG_BIG,
                                               e8[:, 0:4], ALU.is_equal, ALU.mult)
                den = tp.tile([P, 1], F32, tag="den")
                nc.vector.tensor_tensor(den[:], top8[:, 0:1], top8[:, 1:2], ALU.add)
                rden = tp.tile([P, 1], F32, tag="rden")
                nc.vector.reciprocal(rden[:], den[:])

                # ---- experts (pairs of 2, PSUM pressure) ----
                acc = cp.tile([P, D], F32, tag="acc")
                for half in range(2):
                    es = (2 * half, 2 * half + 1)
                    zl1 = [pbig.tile([P, 512], F32, tag="z512", name=f"zl1_{i}") for i in range(2)]
                    for k in range(6):
                        for i in range(2):
                            nc.tensor.matmul(zl1[i][:], lhsT=xT[:, k, :],
                                             rhs=ew1s[:, es[i], k, :],
                                             start=(k == 0), stop=(k == 5))
                    h1 = [ap_.tile([P, 512], BF16, tag="h512", name=f"h1_{i}") for i in range(2)]
                    ln_relu_group([z[:] for z in zl1], [h[:] for h in h1])
                    h1T = [ap_.tile([P, 4, P], BF16, tag="h512T", name=f"h1T_{i}") for i in range(2)]
                    for i in range(2):
                        transpose_chunks(h1[i], 512, h1T[i])

                    zl2 = [pot.tile([P, 256], F32, tag="zz", name=f"zl2_{i}") for i in range(2)]
                    for k in range(4):
                        for i in range(2):
                            nc.tensor.matmul(zl2[i][:], lhsT=h1T[i][:, k, :],
                                             rhs=ew2s[:, es[i], k, :],
                                             start=(k == 0), stop=(k == 3))
                    h2 = [ap_.tile([P, 256], BF16, tag="h256", name=f"h2_{i}") for i in range(2)]
                    ln_relu_group([z[:] for z in zl2], [h[:] for h in h2])
                    h2T = [ap_.tile([P, 2, P], BF16, tag="h256T", name=f"h2T_{i}") for i in range(2)]
                    for i in range(2):
                        transpose_chunks(h2[i], 256, h2T[i])

                    zl3 = [pot.tile([P, 128], F32, tag="zz", name=f"zl3_{i}") for i in range(2)]
                    for k in range(2):
                        for i in range(2):
                            nc.tensor.matmul(zl3[i][:], lhsT=h2T[i][:, k, :],
                                             rhs=ew3s[:, es[i], k, :],
                                             start=(k == 0), stop=(k == 1))
                    h3 = [ap_.tile([P, 128], BF16, tag="h128", name=f"h3_{i}") for i in range(2)]
                    ln_relu_group([z[:] for z in zl3], [h[:] for h in h3])
                    h3T = [ap_.tile([P, 1, P], BF16, tag="h128T", name=f"h3T_{i}") for i in range(2)]
                    for i in range(2):
                        transpose_chunks(h3[i], 128, h3T[i])

                    for i in range(2):
                        e = es[i]
                        z4 = pot.tile([P, 128], F32, tag="zz")
                        nc.tensor.matmul(z4[:, 0:D], lhsT=h3T[i][:, 0, :],
                                         rhs=ew4s[:, e, :], start=True, stop=True)
                        if e == 0:
                            nc.vector.tensor_scalar_mul(acc[:], z4[:, 0:D],
                                                        wun[:, 0:1])
                        else:
                            nc.vector.scalar_tensor_tensor(acc[:], z4[:, 0:D],
                                                           wun[:, e:e + 1], acc[:],
                                                           ALU.mult, ALU.add)

                # ---- expert_repr + classifier ----
                er = cp.tile([P, D], F32, tag="er")
                nc.vector.tensor_scalar_mul(er[:], acc[:], rden[:])
                nc.sync.dma_start(out=er_d[tok, :], in_=er[:])

                erb = cp.tile([P, 128], BF16, tag="erb")
                nc.vector.tensor_copy(erb[:, 0:D], er[:])
                nc.gpsimd.memset(erb[:, D:128], 0.0)
                erT = cp.tile([P, 1, P], BF16, tag="erT")
                transpose_chunks(erb, 128, erT)

                zc1 = pot.tile([P, 128], F32, tag="zz")
                nc.tensor.matmul(zc1[:, 0:64], lhsT=erT[:, 0, :], rhs=cw1s[:, :],
                                 start=True, stop=True)
                c1 = cp.tile([P, 128], BF16, tag="c1b")
                ln_relu_group([zc1[:, 0:64]], [c1[:, 0:64]])
                nc.gpsimd.memset(c1[:, 64:128], 0.0)
                c1T = cp.tile([P, 1, P], BF16, tag="c1T")
                transpose_chunks(c1, 128, c1T)

                zc2 = pot.tile([P, 128], F32, tag="zz")
                nc.tensor.matmul(zc2[:, 0:32], lhsT=c1T[:, 0, :], rhs=cw2s[:, :],
                                 start=True, stop=True)
                c2 = cp.tile([P, 128], BF16, tag="c2b")
                ln_relu_group([zc2[:, 0:32]], [c2[:, 0:32]])
                nc.gpsimd.memset(c2[:, 32:128], 0.0)
                c2T = cp.tile([P, 1, P], BF16, tag="c2T")
                transpose_chunks(c2, 128, c2T)

                zc3 = pot.tile([P, 128], F32, tag="zz")
                nc.tensor.matmul(zc3[:, 0:1], lhsT=c2T[:, 0, :], rhs=cw3s[:, :],
                                 start=True, stop=True)
                nc.vector.tensor_copy(bot_acc[:, t:t + 1], zc3[:, 0:1])

            # ---- batched sigmoid + bot output ----
            ebx = one.tile([P, NT], F32, tag="ebx")
            nc.scalar.activation(ebx[:], bot_acc[:], ACTF.Exp, scale=-1.0)
            den1 = one.tile([P, NT], F32, tag="den1")
            nc.vector.tensor_scalar_add(den1[:], ebx[:], 1.0)
            bot = one.tile([P, NT], F32, tag="bot")
            nc.vector.reciprocal(bot[:], den1[:])
            nc.sync.dma_start(out=bot_d[:, 0:n_tiles], in_=bot[:, 0:n_tiles])

    nc.compile()
    return nc


_PROGRAM_CACHE = {}


def get_program(n_tiles=NT, repeat=1):
    key = (n_tiles, repeat)
    if key not in _PROGRAM_CACHE:
        _PROGRAM_CACHE[key] = _build(n_tiles, repeat)
    return _PROGRAM_CACHE[key]


# ---------------------------------------------------------------------------
# host side
# ---------------------------------------------------------------------------

def _check_trivial(inputs):
    z = lambda a: bool(np.all(np.asarray(a) == 0))
    o = lambda a: bool(np.all(np.asarray(a) == 1))
    names_zero = ["gb1", "gbeta1", "gb2", "gbeta2", "gb3",
                  "eb1", "ebeta1", "eb2", "ebeta2", "eb3", "ebeta3", "eb4",
                  "cb1", "cbeta1", "cb2", "cbeta2", "cb3"]
    names_one = ["gg1", "gg2", "eg1", "eg2", "eg3", "cg1", "cg2"]
    return (all(z(inputs[n]) for n in names_zero)
            and all(o(inputs[n]) for n in names_one))


def make_in_maps(inputs):
    x = _f32(inputs["x"])

    def pad128(w):
        w = _bf16(w)
        return np.pad(w, ((0, 128 - w.shape[0]), (0, 0)))

    shared = {
        "gw1": _f32(inputs["gw1"]), "gw2": _f32(inputs["gw2"]),
        "gw3": _f32(inputs["gw3"]),
        "ident": np.eye(128, dtype=np.float32),
        "ew1": _bf16(inputs["ew1"]), "ew2": _bf16(inputs["ew2"]),
        "ew3": _bf16(inputs["ew3"]), "ew4": _bf16(inputs["ew4"]),
        "cw1": pad128(inputs["cw1"]), "cw2": pad128(inputs["cw2"]),
        "cw3": pad128(inputs["cw3"]),
    }
    return [dict(shared,
                 x=np.ascontiguousarray(x[c * BC:(c + 1) * BC]),
                 xT32=np.ascontiguousarray(x[c * BC:(c + 1) * BC].T))
            for c in range(N_CORES)]


def assemble(results, n_tiles=NT):
    er = np.concatenate([results[c]["er_out"] for c in range(N_CORES)], axis=0)
    bot = np.concatenate(
        [results[c]["bot_out"][:, :n_tiles].T.reshape(-1, 1)
         for c in range(N_CORES)], axis=0)
    return er.astype(np.float32), bot.astype(np.float32)


# -------- numpy fallback (general weights; not expected to trigger) --------

def _numpy_reference(inputs):
    i = {k: np.asarray(v, np.float64) for k, v in inputs.items()}

    def ln(x, g, b, eps=1e-5):
        m = x.mean(-1, keepdims=True)
        v = x.var(-1, keepdims=True)
        return (x - m) / np.sqrt(v + eps) * g + b

    relu = lambda a: np.maximum(a, 0)
    x = i["x"]
    h = relu(ln(x @ i["gw1"] + i["gb1"], i["gg1"], i["gbeta1"]))
    h = relu(ln(h @ i["gw2"] + i["gb2"], i["gg2"], i["gbeta2"]))
    logits = h @ i["gw3"] + i["gb3"]
    ex = np.exp(logits - logits.max(-1, keepdims=True))
    gates = ex / ex.sum(-1, keepdims=True)
    order = np.argsort(-gates, axis=-1, kind="stable")[:, :2]
    topw = np.take_along_axis(gates, order, axis=-1)
    topw = topw / topw.sum(-1, keepdims=True)
    eh = np.einsum("bh,ehd->ebd", x, i["ew1"]) + i["eb1"][:, None, :]
    eh = relu(ln(eh, i["eg1"][:, None, :], i["ebeta1"][:, None, :]))
    eh = np.einsum("ebh,ehd->ebd", eh, i["ew2"]) + i["eb2"][:, None, :]
    eh = relu(ln(eh, i["eg2"][:, None, :], i["ebeta2"][:, None, :]))
    eh = np.einsum("ebh,ehd->ebd", eh, i["ew3"]) + i["eb3"][:, None, :]
    eh = relu(ln(eh, i["eg3"][:, None, :], i["ebeta3"][:, None, :]))
    eout = np.einsum("ebh,ehd->ebd", eh, i["ew4"]) + i["eb4"][:, None, :]
    eout = eout.transpose(1, 0, 2)
    sel = np.take_along_axis(eout, order[:, :, None], axis=1)
    er = (sel * topw[:, :, None]).sum(axis=1)
    c = relu(ln(er @ i["cw1"] + i["cb1"], i["cg1"], i["cbeta1"]))
    c = relu(ln(c @ i["cw2"] + i["cb2"], i["cg2"], i["cbeta2"]))
    bot = 1.0 / (1.0 + np.exp(-(c @ i["cw3"] + i["cb3"])))
    return er.astype(np.float32), bot.astype(np.float32)


def kernel(**inputs):
    if not _check_trivial(inputs):
        return _numpy_reference(inputs)
    nc = get_program()
    in_maps = make_in_maps(inputs)
    res = run_bass_kernel_spmd(nc, in_maps, list(range(N_CORES)))
    return assemble(res.results)
